# revision 3
# baseline (speedup 1.0000x reference)
"""DeepSeekMoE layer on 8 TRN2 NeuronCores — expert-parallel, fused MLP.

Reference computation (per token):
    shared = silu(x @ ws1) @ ws2
    router: softmax(x @ w_router) -> top-2 -> renormalize -> gates
    routed = sum_{e in top2} gate_e * silu(x @ w1[e]) @ w2[e]
    out    = shared + routed

Sharding: expert-parallel. Core e receives the (padded to 128) bucket of all
token rows routed to expert e (capacity C), plus a 1/8 slice of all tokens
for the shared expert (S rows), packed into one [128, 16, C+S] input. Routing
(softmax/top-k) and the dispatch/combine permutations run on the host; all
GEMMs + SiLU + gate scaling run on device.

Device kernel: for each 512-token tile, both GEMMs run back to back with the
intermediate h = silu(x @ w1) kept in SBUF (no DRAM round trip). All matmul
operands are bf16 (same 1 cycle/row PE rate as fp32r, half the DMA traffic);
accumulation is fp32 in PSUM. Both weight matrices of the active expert stay
SBUF-resident; the four per-rep weight sets rotate through 3 SBUF slots, with
each load emitted at the program point where its WAR hazard clears so the
issuing queue never head-blocks.
"""

import numpy as np
import ml_dtypes

import concourse.mybir as mybir
import concourse.tile as tile
from concourse import bacc
from concourse.bass_utils import run_bass_kernel_spmd

H = 2048          # hidden
I = 1408          # moe intermediate
E = 8             # routed experts == n cores
NCORES = 8
RT = 512          # token tile (free dim of pass1 / partition chunks of pass2)
KH = H // 128     # 16 k-tiles over hidden
KI = 11           # 11 k-tiles over intermediate
F32 = mybir.dt.float32
BF16 = mybir.dt.bfloat16

_BUILD_CACHE: dict = {}


def _tiles(ncols):
    """R-tile (offset, size) list: 512-tiles plus an optional 128/256/384 tail."""
    out, off = [], 0
    while ncols - off >= RT:
        out.append((off, RT))
        off += RT
    if ncols - off:
        assert (ncols - off) % 128 == 0
        out.append((off, ncols - off))
    return out


def _emit_phase(nc, pools, w1_sb, w2_sb, xt, y, scale_sb, col_off, ncols, ph,
                post_p1_hook=None):
    """One expert phase: y[rows] = silu(x[rows] @ w1) @ w2 * gate[rows].

    Per R-tile: pass1 accumulates hT [I x R] (I on partitions) over 16
    hidden k-slices in two PSUM groups (6+5 banks), evicting through SiLU
    into an SBUF-resident bf16 h tile; pass2 immediately consumes h as lhsT
    to produce y [R x H] (tokens on partitions) in chunks of 8 PSUM banks,
    evicted with the per-token gate scale into bf16 and stored.

    post_p1_hook() is invoked right after the LAST tile's pass1 evictions —
    the point where w1_sb's final read is in flight, so a weight DMA emitted
    there into w1_sb's rotation slot never blocks the queue on its WAR.
    """
    in_pool, psum_pool = pools
    tiles = _tiles(ncols)
    hook_ret = None
    for ti, (off, sz) in enumerate(tiles):
        g0 = col_off + off
        cs = slice(g0, g0 + sz)
        x_sb = in_pool.tile([128, KH, RT], BF16, tag="x", bufs=2,
                            name=f"x_{ph}_{ti}")
        nc.sync.dma_start(x_sb[:, 0:8, :sz], xt[:, 0:8, cs])
        nc.sync.dma_start(x_sb[:, 8:16, :sz], xt[:, 8:16, cs])
        h_sb = in_pool.tile([128, KI, RT], BF16, tag="h", bufs=1,
                            name=f"h_{ph}_{ti}")

        # ---- pass1: hT = silu(w1.T @ xT), I on partitions ----
        # one full accumulation chain per PSUM bank (no bank hopping: ~14ns
        # cheaper per matmul on HW), eviction trails each chain on Act
        for i in range(KI):
            ps = psum_pool.tile([128, sz], F32, tag="ps",
                                name=f"ps1_{ph}_{ti}_{i}")
            for k in range(KH):
                nc.tensor.matmul(
                    ps[:], w1_sb[:, k, i * 128:(i + 1) * 128],
                    x_sb[:, k, :sz], start=(k == 0), stop=(k == KH - 1))
            nc.scalar.activation(h_sb[:, i, :sz], ps[:],
                                 mybir.ActivationFunctionType.Silu)
        if ti == len(tiles) - 1 and post_p1_hook is not None:
            hook_ret = post_p1_hook()

        # ---- pass2: y = (hT.T @ w2) * gate, tokens on partitions ----
        for c in range(sz // 128):
            m = g0 // 128 + c
            yb = in_pool.tile([128, H], BF16, tag="y", bufs=4,
                              name=f"yb_{ph}_{ti}_{c}")
            for hb in range(H // 512):
                ps = psum_pool.tile([128, 512], F32, tag="ps",
                                    name=f"ps2_{ph}_{ti}_{c}_{hb}")
                for i in range(KI):
                    nc.tensor.matmul(
                        ps[:], h_sb[:, i, c * 128:(c + 1) * 128],
                        w2_sb[:, i, hb * 512:(hb + 1) * 512],
                        start=(i == 0), stop=(i == KI - 1))
                nc.vector.tensor_scalar_mul(
                    yb[:, hb * 512:(hb + 1) * 512], ps[:],
                    scale_sb[:, m:m + 1])
            nc.gpsimd.dma_start(y[:, m, :], yb[:])
    return hook_ret


def build(C, S, debug=False, use_silu=True, reps=1):
    """Build the per-core Bass module. C: expert capacity, S: shared rows.

    reps>1 repeats the whole computation in one NEFF (timing use only)."""
    assert C % 128 == 0 and S % RT == 0
    R = C + S
    nc = bacc.Bacc(None, target_bir_lowering=False, debug=debug)
    with tile.TileContext(nc) as tc:
        with tc.tile_pool(name="dram", bufs=1, space="DRAM") as dram:
            xt = dram.tile((128, KH, R), BF16, kind="ExternalInput", name="xt", uniquify=False)
            w1e = dram.tile((128, KH, I), BF16, kind="ExternalInput", name="w1e", uniquify=False)
            w2e = dram.tile((128, KI, H), BF16, kind="ExternalInput", name="w2e", uniquify=False)
            ws1 = dram.tile((128, KH, I), BF16, kind="ExternalInput", name="ws1", uniquify=False)
            ws2 = dram.tile((128, KI, H), BF16, kind="ExternalInput", name="ws2", uniquify=False)
            gate = dram.tile((128, R // 128), F32, kind="ExternalInput", name="gate", uniquify=False)
            y = dram.tile((128, R // 128, H), BF16, kind="ExternalOutput", name="y", uniquify=False)

            with (
                tc.tile_pool(name="wpool", bufs=3) as wpool,
                tc.tile_pool(name="inpool", bufs=2) as in_pool,
                tc.tile_pool(name="psum", bufs=8, space="PSUM") as psum_pool,
                tc.tile_pool(name="const", bufs=1) as const_pool,
            ):
                pools = (in_pool, psum_pool)
                scale_sb = const_pool.tile([128, R // 128], F32, name="scale_sb")
                nc.sync.dma_start(scale_sb[:], gate[:])

                def load_w(dram_w, nk, nm):
                    # weight loads ride the Activation queue, k-sliced so the
                    # PE can consume slices as they stream in
                    t = wpool.tile([128, nk, dram_w.shape[2]], BF16, tag="w",
                                   name=nm)
                    for k in range(nk):
                        nc.scalar.dma_start(t[:, k, :], dram_w[:, k, :])
                    return t

                # 3-slot rotation: per rep the four sets allocate in order
                # w1e, w2e, ws1, ws2 -> slots cycle 0,1,2,0,1,2..., and every
                # load is emitted exactly where its WAR on the evicted set
                # clears (see hooks below).
                w1s = load_w(w1e, KH, "w1s_0")
                w2s = load_w(w2e, KI, "w2s_0")
                for rep in range(reps):
                    ws1s = load_w(ws1, KH, f"ws1s_{rep}")
                    # routed phase; after its last pass1 (w1s dead) load ws2
                    ws2s = _emit_phase(
                        nc, pools, w1s, w2s, xt, y, scale_sb, 0, C,
                        f"r{rep}",
                        post_p1_hook=lambda: load_w(ws2, KI, f"ws2s_{rep}"))
                    # shared phase; at start w2s is dead -> next rep's w1;
                    # after its last pass1 ws1s is dead -> next rep's w2
                    w1s = (load_w(w1e, KH, f"w1s_{rep + 1}")
                           if rep + 1 < reps else None)
                    w2s = _emit_phase(
                        nc, pools, ws1s, ws2s, xt, y, scale_sb, C, S,
                        f"s{rep}",
                        post_p1_hook=(
                            (lambda: load_w(w2e, KI, f"w2s_{rep + 1}"))
                            if rep + 1 < reps else None))

    nc.compile()
    return nc


def _get_built(C, S):
    key = (C, S)
    if key not in _BUILD_CACHE:
        _BUILD_CACHE[key] = build(C, S)
    return _BUILD_CACHE[key]


def _to_kxm_layout(a):
    """[K, M] -> [128, K/128, M] with logical row k at (k%128, k//128)."""
    k, m_ = a.shape
    return np.ascontiguousarray(a.reshape(k // 128, 128, m_).transpose(1, 0, 2))


def route_and_dispatch(xf, w_router):
    """Host router: returns (sorted token ids, gates, per-expert offsets, capacity)."""
    T = xf.shape[0]
    logits = xf @ w_router                       # [T, E]
    order = np.argsort(-logits, axis=1, kind="stable")[:, :2]
    mx = logits.max(axis=1, keepdims=True)
    p = np.exp(logits - mx)
    p /= p.sum(axis=1, keepdims=True)
    tk = np.take_along_axis(p, order, axis=1)    # [T, 2]
    g = tk / tk.sum(axis=1, keepdims=True)

    pe = order.ravel()                           # expert id per (token, slot) pair
    ptok = np.repeat(np.arange(T, dtype=np.int64), 2)
    pg = g.astype(np.float32).ravel()
    perm = np.argsort(pe, kind="stable")
    stok, sg = ptok[perm], pg[perm]
    counts = np.bincount(pe, minlength=E)
    offs = np.zeros(E + 1, dtype=np.int64)
    np.cumsum(counts, out=offs[1:])
    C = max(512, int(-(-counts.max() // 128) * 128))
    return stok, sg, offs, C


def prepare(x, w_shared1, w_shared2, w1, w2, w_router):
    """Host-side routing + dispatch. Returns (in_maps, meta)."""
    x = np.asarray(x, dtype=np.float32)
    w_router = np.asarray(w_router, dtype=np.float32)

    B, Sq, _ = x.shape
    T = B * Sq
    S = T // NCORES                              # shared-expert rows per core
    xf = x.reshape(T, H)

    stok, sg, offs, C = route_and_dispatch(xf, w_router)
    R = C + S

    bf = ml_dtypes.bfloat16
    xb = xf.astype(bf)
    w1b = np.asarray(w1, dtype=np.float32).astype(bf)
    w2b = np.asarray(w2, dtype=np.float32).astype(bf)
    ws1_l = _to_kxm_layout(np.asarray(w_shared1, np.float32).astype(bf))
    ws2_l = _to_kxm_layout(np.asarray(w_shared2, np.float32).astype(bf))

    in_maps = []
    for e in range(NCORES):
        toks = stok[offs[e]:offs[e + 1]]
        n = len(toks)
        xd = np.zeros((R, H), bf)
        xd[:n] = xb[toks]
        xd[C:] = xb[e * S:(e + 1) * S]
        gate_v = np.zeros(R, np.float32)
        gate_v[:n] = sg[offs[e]:offs[e + 1]]
        gate_v[C:] = 1.0
        in_maps.append({
            "xt": np.ascontiguousarray(xd.reshape(R, KH, 128).transpose(2, 1, 0)),
            "w1e": _to_kxm_layout(w1b[e]),
            "w2e": _to_kxm_layout(w2b[e]),
            "ws1": ws1_l,
            "ws2": ws2_l,
            "gate": np.ascontiguousarray(gate_v.reshape(R // 128, 128).T),
        })

    meta = (B, Sq, T, S, C, stok, offs)
    return in_maps, meta


def combine(results, meta):
    """Host-side gather/unshard of per-core outputs to the full output."""
    B, Sq, T, S, C, stok, offs = meta
    out = np.zeros((T, H), np.float32)
    for e in range(NCORES):
        toks = stok[offs[e]:offs[e + 1]]
        yp = np.asarray(results[e]["y"], dtype=np.float32
                        ).transpose(1, 0, 2).reshape(C + S, H)
        out[toks] += yp[:len(toks)]
        out[e * S:(e + 1) * S] += yp[C:]
    return out.reshape(B, Sq, H)


def kernel(x, w_shared1, w_shared2, w1, w2, w_router):
    in_maps, meta = prepare(x, w_shared1, w_shared2, w1, w2, w_router)
    C, S = meta[4], meta[3]
    nc = _get_built(C, S)
    res = run_bass_kernel_spmd(nc, in_maps, core_ids=list(range(NCORES)))
    return combine(res.results, meta)
